# revision 18
# baseline (speedup 1.0000x reference)
"""Trainium2 Bass kernel for nn_Loss_34230889349355 (superquadric loss).

Data-parallel over B=8 (one batch/core).  Changes vs v1:
- K=5 matmul (lhs rows [pc,1,||pc||^2], rhs rows [-2X',||X'||^2,1]) so PSUM
  holds true squared distances; per-tile relu/bias pass eliminated.
- Balanced ACT/DVE drain, alternating per tile: ACT full-copies 6 (even
  tiles) or 7 (odd tiles) prims/tile to bf16, DVE TR-mins the rest straight
  from PSUM; bf16 folds at DVE 2x; tail TRs batched per 4 tiles.
- Sampling act-ops batched (sin x4 / ln x4 / exp x4) -> 3 act-table loads
  instead of 9.
- Transforms drained by ACT copies into an axis-major bf16 tile; cuboid
  runs on bf16 (DVE 2x / ACT), interleaved with the main loop.
"""

import numpy as np

B, N, P, S = 8, 4096, 16, 200
T = N // 128            # 32 n-tiles
PS = P * S              # 3200 distance columns per n-row

_CACHE = {}


def _build():
    import concourse.bacc as bacc
    import concourse.tile as tile
    import concourse.bass as bass
    from concourse import mybir

    f32 = mybir.dt.float32
    f32r = mybir.dt.float32r
    bf16 = mybir.dt.bfloat16
    ALU = mybir.AluOpType
    ACT = mybir.ActivationFunctionType
    AX = mybir.AxisListType

    nc = bacc.Bacc(
        trn_type="TRN2",
        target_bir_lowering=False,
        debug=False,
        enable_asserts=False,
        num_devices=8,
    )

    pc_d = nc.dram_tensor("pc", [N, 3], f32, kind="ExternalInput")
    nr_d = nc.dram_tensor("normals", [N, 3], f32, kind="ExternalInput")
    tr_d = nc.dram_tensor("trans", [P, 3], f32, kind="ExternalInput")
    ro_d = nc.dram_tensor("rotate", [P, 3, 3], f32, kind="ExternalInput")
    sc_d = nc.dram_tensor("scale", [P, 3], f32, kind="ExternalInput")
    ep_d = nc.dram_tensor("shape_eps", [P, 2], f32, kind="ExternalInput")
    et_d = nc.dram_tensor("etas", [P, S], f32, kind="ExternalInput")
    om_d = nc.dram_tensor("omegas", [P, S], f32, kind="ExternalInput")
    as_d = nc.dram_tensor("assign_matrix", [N, P], f32, kind="ExternalInput")
    out_d = nc.dram_tensor("out", [128, 18], f32, kind="ExternalOutput")

    def dap(tns, ap, offset=0):
        return bass.AP(tensor=tns, offset=offset, ap=ap)

    with tile.TileContext(nc) as tc:
        with (
            tc.tile_pool(name="consts", bufs=1) as cp,
            tc.tile_pool(name="samp", bufs=1) as sp,
            tc.tile_pool(name="work", bufs=4) as wp,
            tc.tile_pool(name="tail", bufs=2) as gp,
            tc.tile_pool(name="psum", bufs=2, space="PSUM") as pp,
        ):
            # ---------------- DMAs (small/critical first) ----------------
            # ones source + const biases (tiny DVE memsets, run at t=0)
            czero = cp.tile([128, 3], f32)
            nc.vector.memset(czero[:, 0:1], 0.0)
            nc.vector.memset(czero[:, 1:2], float(np.pi / 2))
            nc.vector.memset(czero[:, 2:3], 1.0)
            nc.const_aps.aps[(f32, 0.0)] = czero[:, 0:1]
            nc.const_aps.aps[(f32, float(np.pi / 2))] = czero[:, 1:2]

            R4 = cp.tile([4, 3, P], f32r)
            nc.sync.dma_start(out=R4[0:3, :, :],
                              in_=dap(ro_d, [[3, 3], [1, 3], [9, P]]).bitcast(f32r))
            tbT = cp.tile([3, P], f32)
            nc.sync.dma_start(out=tbT, in_=dap(tr_d, [[1, 3], [3, P]]))
            etas = cp.tile([P, S], f32)
            nc.sync.dma_start(out=etas, in_=et_d.ap())
            omegas = cp.tile([P, S], f32)
            nc.sync.dma_start(out=omegas, in_=om_d.ap())
            pc_nt = cp.tile([128, T, 3], f32)
            nc.gpsimd.dma_start(out=pc_nt, in_=dap(pc_d, [[3, 128], [128 * 3, T], [1, 3]]))
            ecols = cp.tile([P, 2], f32)
            nc.sync.dma_start(out=ecols, in_=ep_d.ap())
            acols = cp.tile([P, 3], f32)
            nc.sync.dma_start(out=acols, in_=sc_d.ap())
            tcols = cp.tile([P, 3], f32)
            nc.sync.dma_start(out=tcols, in_=tr_d.ap())
            Rcols = cp.tile([P, 9], f32)
            nc.sync.dma_start(out=Rcols, in_=ro_d.ap().rearrange("p a b -> p (a b)"))
            SCtmp = cp.tile([128, P, 3], f32)
            nc.sync.dma_start(out=SCtmp, in_=dap(sc_d, [[0, 128], [1, P * 3]]))

            pc5T = cp.tile([5, N], f32r)
            nr4T = cp.tile([4, N], f32r)
            for ch in range(4):
                nc.sync.dma_start(
                    out=pc5T[0:3, 1024 * ch: 1024 * (ch + 1)],
                    in_=dap(pc_d, [[1, 3], [3, 1024]], offset=3 * 1024 * ch).bitcast(f32r))
            for ch in range(4):
                nc.sync.dma_start(
                    out=nr4T[0:3, 1024 * ch: 1024 * (ch + 1)],
                    in_=dap(nr_d, [[1, 3], [3, 1024]], offset=3 * 1024 * ch).bitcast(f32r))
            # ones rows via broadcast DMA off czero col 2 (keeps DVE free)
            ones_t = cp.tile([16, 256], f32)
            nc.vector.memset(ones_t, 1.0)
            nc.gpsimd.dma_start(out=pc5T.bitcast(f32)[3:4, :], in_=ones_t)
            nc.gpsimd.dma_start(out=nr4T.bitcast(f32)[3:4, :], in_=ones_t)

            A_all = cp.tile([128, T, P], f32)
            nc.gpsimd.dma_start(out=A_all, in_=dap(as_d, [[P, 128], [128 * P, T], [1, P]]))

            SCf = cp.tile([128, T, 3, P], f32)
            SC = cp.tile([128, T, 3, P], bf16)

            # ||pc_n||^2 -> row 4 of pc5T (col n = t*128 + nr), DRAM roundtrip
            pcsq = cp.tile([128, T, 3], f32)
            nc.vector.tensor_tensor(pcsq, pc_nt, pc_nt, ALU.mult)
            pcn2 = cp.tile([128, T], f32)
            nc.vector.tensor_reduce(pcn2, pcsq, axis=AX.X, op=ALU.add)
            scr_d = nc.dram_tensor("pcn2_scratch", [N], f32, kind="Internal")
            nc.gpsimd.dma_start(out=dap(scr_d, [[1, 128], [128, T]]), in_=pcn2)
            nc.gpsimd.dma_start(out=pc5T.bitcast(f32)[4:5, :],
                              in_=dap(scr_d, [[N, 1], [1, N]]))

            # ---- R4 row 3 + transform matmuls (PE early; drains emitted later) ----
            prod = cp.tile([3, 3, P], f32r)
            for i in range(3):
                nc.vector.tensor_tensor(prod[:, i: i + 1, :], R4[0:3, i: i + 1, :],
                                        tbT.unsqueeze(1), ALU.mult)
            negones3 = cp.tile([3, 1], f32r)
            nc.vector.memset(negones3.bitcast(f32), -1.0)
            rpt = pp.tile([128, 2048], f32, tag="dps", name="rpt")
            nc.tensor.matmul(rpt[0:1, 0:48], negones3,
                             prod.rearrange("a b c -> a (b c)"), start=True, stop=True)
            row3tmp = cp.tile([1, 48], f32r)
            nc.scalar.copy(row3tmp, rpt[0:1, 0:48])
            nc.sync.dma_start(out=R4[3:4, :, :],
                              in_=row3tmp.rearrange("a (i p) -> a i p", i=3))
            pcnI = cp.tile([128, 2, T, 3, P], bf16)   # [:,0]=pcI, [:,1]=nI
            R4f = R4.rearrange("a b c -> a (b c)")
            tf_pts = []

            def emit_transform_mms(half):
                tf = pp.tile([128, 2048], f32, tag="dps", name="tf")
                tfv = tf.rearrange("n (j x) -> n j x", j=16)
                for j in range(16):
                    t = 16 * half + j
                    nc.tensor.matmul(tfv[:, j, 0:48],
                                     pc5T[0:4, 128 * t: 128 * (t + 1)],
                                     R4f, start=True, stop=True)
                    nc.tensor.matmul(tfv[:, j, 64:112],
                                     nr4T[:, 128 * t: 128 * (t + 1)],
                                     R4f, start=True, stop=True)
                tf_pts.append(tfv)

            # ---------------- sampling (batched act tables: 3 loads) ----------
            vals4 = sp.tile([P, 4, S], f32)      # ce, se, co, so
            nc.scalar.activation(vals4[:, 0, :], etas, ACT.Sin, bias=float(np.pi / 2))
            nc.scalar.activation(vals4[:, 1, :], etas, ACT.Sin)
            nc.scalar.activation(vals4[:, 2, :], omegas, ACT.Sin, bias=float(np.pi / 2))
            nc.scalar.activation(vals4[:, 3, :], omegas, ACT.Sin)
            av4 = sp.tile([P, 4, S], f32)
            nc.scalar.activation(av4, vals4, ACT.Abs)
            nc.scalar.activation(av4, av4, ACT.Ln)
            e1 = ecols[:, 0:1]
            e2 = ecols[:, 1:2]
            nc.vector.tensor_scalar(av4[:, 0:2, :], av4[:, 0:2, :], e1, None, ALU.mult)
            nc.vector.tensor_scalar(av4[:, 2:4, :], av4[:, 2:4, :], e2, None, ALU.mult)
            nc.scalar.activation(av4, av4, ACT.Exp)
            sg4 = sp.tile([P, 4, S], f32)
            nc.scalar.activation(sg4, vals4, ACT.Sign)

            def clampc(mi1, mi2, a_col, nm):
                # sign(v1)sign(v2) * max(a*|f1|*|f2|, 1e-6); av4 holds |f|^e
                m = sp.tile([P, S], f32, tag=nm + "_m", name=nm + "_m")
                if mi2 is not None:
                    nc.vector.tensor_tensor(m, av4[:, mi1, :], av4[:, mi2, :], ALU.mult)
                    nc.vector.tensor_scalar(m, m, a_col, None, ALU.mult)
                else:
                    nc.vector.tensor_scalar(m, av4[:, mi1, :], a_col, None, ALU.mult)
                nc.vector.tensor_scalar(m, m, 1e-6, None, ALU.max)
                if mi2 is not None:
                    s = sp.tile([P, S], f32, tag=nm + "_s", name=nm + "_s")
                    nc.vector.tensor_tensor(s, sg4[:, mi1, :], sg4[:, mi2, :], ALU.mult)
                    nc.vector.tensor_tensor(m, m, s, ALU.mult)
                else:
                    nc.vector.tensor_tensor(m, m, sg4[:, mi1, :], ALU.mult)
                return m

            xc = clampc(0, 2, acols[:, 0:1], "xc")
            yc = clampc(0, 3, acols[:, 1:2], "yc")
            zc = clampc(1, None, acols[:, 2:3], "zc")

            # world frame: X'' = -2(R X + t); rhs5 rows 0-2 = X''_i,
            # row 3 = ||X''||^2/4 (= ||X'||^2), row 4 = ones
            R2 = sp.tile([P, 9], f32)
            nc.vector.tensor_scalar(R2, Rcols, -2.0, None, ALU.mult)
            t2 = sp.tile([P, 3], f32)
            nc.vector.tensor_scalar(t2, tcols, -2.0, None, ALU.mult)

            rhs5 = cp.tile([5, PS], f32r)
            nc.gpsimd.dma_start(out=rhs5.bitcast(f32)[4:5, :], in_=ones_t[:, 0:200])
            Xp = []
            for i in range(3):
                u = sp.tile([P, S], f32r, tag=f"xp{i}", name=f"xp{i}")
                nc.vector.tensor_scalar(u, xc, R2[:, 3 * i + 0: 3 * i + 1], None, ALU.mult)
                nc.vector.scalar_tensor_tensor(u, yc, R2[:, 3 * i + 1: 3 * i + 2], u,
                                               ALU.mult, ALU.add)
                nc.vector.scalar_tensor_tensor(u, zc, R2[:, 3 * i + 2: 3 * i + 3], u,
                                               ALU.mult, ALU.add)
                nc.vector.tensor_scalar(u, u, t2[:, i: i + 1], None, ALU.add)
                Xp.append(u)
            sq0 = sp.tile([P, S], f32r)
            nc.vector.tensor_tensor(sq0, Xp[0], Xp[0], ALU.mult)
            sq1 = sp.tile([P, S], f32r)
            nc.vector.tensor_tensor(sq1, Xp[1], Xp[1], ALU.mult)
            nc.vector.tensor_tensor(sq0, sq0, sq1, ALU.add)
            nc.vector.tensor_tensor(sq1, Xp[2], Xp[2], ALU.mult)
            nc.vector.tensor_tensor(sq0, sq0, sq1, ALU.add)
            nc.vector.tensor_scalar(sq0, sq0, 0.25, None, ALU.mult)
            for i, src_t in enumerate(Xp + [sq0]):
                nc.gpsimd.dma_start(
                    out=rhs5[i: i + 1, :].rearrange("a (p s) -> a p s", p=P),
                    in_=src_t)

            nc.vector.tensor_copy(SCf[:, 0, :, :], SCtmp.rearrange("n p i -> n i p"))
            _w = 1
            while _w < T:
                _c = min(_w, T - _w)
                nc.vector.tensor_copy(SCf[:, _w:_w + _c, :, :], SCf[:, 0:_c, :, :])
                _w += _c
            nc.vector.tensor_copy(SC, SCf)

            def emit_transform_drains(half):
                tfv = tf_pts[half]
                nc.scalar.copy(
                    pcnI[:, 0, 16 * half: 16 * half + 16, :, :],
                    tfv[:, :, 0:48].rearrange("n t (i p) -> n t i p", i=3))
                nc.scalar.copy(
                    pcnI[:, 1, 16 * half: 16 * half + 16, :, :],
                    tfv[:, :, 64:112].rearrange("n t (i p) -> n t i p", i=3))

            # ---------------- cuboid (batched bf16, interleaved) --------------
            pcI = pcnI[:, 0]   # [128, T, 3, P]
            nI = pcnI[:, 1]
            cub = cp.tile([128, T, P], f32)
            cbt = {}

            def cb(nm, shape=None, dtype=bf16):
                if nm not in cbt:
                    cbt[nm] = cp.tile(shape or [128, T, 3, P], dtype,
                                      tag="cb_" + nm, name="cb_" + nm)
                return cbt[nm]

            def emit_cuboid(step):
                if step == 0:
                    ax = cb("ax")
                    nc.scalar.activation(ax, pcI, ACT.Abs)
                    w1 = cb("w1")
                    nc.vector.tensor_tensor(w1, ax, SC, ALU.subtract)
                elif step == 1:
                    w1 = cbt["w1"]
                    nc.scalar.activation(w1, w1, ACT.Relu)
                    ee = cb("ee")
                    nc.scalar.activation(ee, w1, ACT.Square)
                elif step == 2:
                    gg = cb("gg")
                    nc.vector.tensor_scalar(gg, nI, 0.0, None, ALU.is_gt)
                    mm_ = cb("mm")
                    nc.vector.tensor_tensor(mm_, gg, pcI, ALU.mult)
                elif step == 3:
                    u = cb("u")
                    nc.vector.scalar_tensor_tensor(u, cbt["mm"], 2.0, pcI, ALU.mult,
                                                   ALU.subtract)
                    nc.vector.tensor_tensor(u, u, SC, ALU.subtract)
                elif step == 4:
                    qq = cb("qq")
                    nc.scalar.activation(qq, cbt["u"], ACT.Square)
                    dd = cb("dd")
                    nc.vector.tensor_tensor(dd, qq, cbt["ee"], ALU.subtract)
                elif step == 5:
                    tA = cb("tA")
                    nc.scalar.activation(tA, nI, ACT.Abs)
                    E = cb("E", [128, T, P])
                    nc.vector.tensor_tensor(E, cbt["ee"][:, :, 0, :],
                                            cbt["ee"][:, :, 1, :], ALU.add)
                    nc.vector.tensor_tensor(E, E, cbt["ee"][:, :, 2, :], ALU.add)
                elif step == 6:
                    tA = cbt["tA"]
                    c1 = cb("c1", [128, T, P], mybir.dt.uint8)
                    nc.vector.tensor_tensor(c1, tA[:, :, 0, :], tA[:, :, 1, :], ALU.is_ge)
                    t1 = cb("t1", [128, T, P])
                    nc.vector.tensor_tensor(t1, tA[:, :, 0, :], tA[:, :, 1, :], ALU.max)
                    c2 = cb("c2", [128, T, P], mybir.dt.uint8)
                    nc.vector.tensor_tensor(c2, t1, tA[:, :, 2, :], ALU.is_ge)
                elif step == 7:
                    dd = cbt["dd"]
                    d1 = cb("d1", [128, T, P])
                    nc.vector.select(d1, cbt["c1"], dd[:, :, 0, :], dd[:, :, 1, :])
                    dsel = cb("dsel", [128, T, P])
                    nc.vector.select(dsel, cbt["c2"], d1, dd[:, :, 2, :])
                    nc.vector.tensor_tensor(cub, cbt["E"], dsel, ALU.add)

            # ---------------- main loop ----------------
            minn = cp.tile([128, T, P], f32)
            G4_cur = [None]

            state = {}

            def emit_folds(t):
                even = (t % 2 == 0)
                sbA, F1, F2, G4 = state[t]
                if even:
                    nc.vector.tensor_tensor(
                        F1[:, :, 0:6, :], sbA[:, :, 0:6, 0:100],
                        sbA[:, :, 0:6, 100:200], ALU.min)
                    nc.vector.tensor_tensor(F2[:, :, 0:6, :], F1[:, :, 0:6, 0:50],
                                            F1[:, :, 0:6, 50:100], ALU.min)
                else:
                    nc.vector.tensor_tensor(F1[:, 0, :, :], sbA[:, 0, :, 0:100],
                                            sbA[:, 0, :, 100:200], ALU.min)
                    nc.vector.tensor_tensor(F1[:, 1, 0:6, :], sbA[:, 1, 0:6, 0:100],
                                            sbA[:, 1, 0:6, 100:200], ALU.min)
                    nc.vector.tensor_tensor(F2, F1[:, :, :, 0:50], F1[:, :, :, 50:100],
                                            ALU.min)
                nc.vector.tensor_tensor(G4[:, t % 4], F2[:, :, :, 0:25],
                                        F2[:, :, :, 25:50], ALU.min)
                del state[t]
                if t % 4 == 3 or t == T - 2:
                    g = t // 4
                    nc.vector.tensor_reduce(
                        minn[:, 4 * g: 4 * g + 4: 2, :]
                            .rearrange("n t (h p) -> n t h p", h=2)[:, :, :, 0:6],
                        G4[:, 0:4:2, :, 0:6, :], axis=AX.X, op=ALU.min)
                    if t % 4 == 3:
                        nc.vector.tensor_reduce(
                            minn[:, 4 * g + 1: 4 * g + 4: 2, 0:8],
                            G4[:, 1:4:2, 0, :, :], axis=AX.X, op=ALU.min)
                        nc.vector.tensor_reduce(
                            minn[:, 4 * g + 1: 4 * g + 4: 2, 8:14],
                            G4[:, 1:4:2, 1, 0:6, :], axis=AX.X, op=ALU.min)
                    else:
                        nc.vector.tensor_reduce(
                            minn[:, 4 * g + 1, 0:8].unsqueeze(1),
                            G4[:, 1:2, 0, :, :], axis=AX.X, op=ALU.min)
                        nc.vector.tensor_reduce(
                            minn[:, 4 * g + 1, 8:14].unsqueeze(1),
                            G4[:, 1:2, 1, 0:6, :], axis=AX.X, op=ALU.min)

            def main_tile(t):
                last = (t == T - 1)
                even = (t % 2 == 0)
                if not last:
                    sbA = wp.tile([128, 2, 8, 200], bf16, tag="sbA", name="sbA")
                    F1 = wp.tile([128, 2, 8, 100], bf16, tag="F1", name="F1")
                    F2 = wp.tile([128, 2, 8, 50], bf16, tag="F2", name="F2")
                if t % 4 == 0:
                    G4_cur[0] = gp.tile([128, 4, 2, 8, 25], bf16, tag="G4", name="G4")
                if not last:
                    state[t] = (sbA, F1, F2, G4_cur[0])
                for h in range(2):
                    dt = pp.tile([128, 2048], f32, tag="dps", name="dt")
                    dv = dt.rearrange("n (a x) -> n a x", a=4)
                    for q in range(4):
                        nc.tensor.matmul(
                            dv[:, q, 0:400], pc5T[:, 128 * t: 128 * (t + 1)],
                            rhs5[:, 1600 * h + 400 * q: 1600 * h + 400 * (q + 1)],
                            start=True, stop=True)
                    if last:
                        nc.vector.tensor_reduce(
                            minn[:, t, 8 * h: 8 * h + 8]
                                .rearrange("n (b p) -> n b p", b=4),
                            dv[:, :, 0:400].rearrange("n b (p s) -> n b p s", p=2),
                            axis=AX.X, op=ALU.min)
                        continue
                    nacts = 4 if (not even and h == 0) else 3
                    k = 2 * nacts
                    nc.scalar.copy(
                        sbA[:, h, 0:k, :].rearrange("n (b p) s -> n b p s", b=nacts),
                        dv[:, 0:nacts, 0:400].rearrange("n b (p s) -> n b p s", p=2))
                    if nacts == 3:
                        nc.vector.tensor_reduce(
                            minn[:, t, 8 * h + 6: 8 * h + 8],
                            dv[:, 3, 0:400].rearrange("n (p s) -> n p s", p=2),
                            axis=AX.X, op=ALU.min)
                if t > 0:
                    emit_folds(t - 1)

            emit_transform_mms(0)
            emit_transform_drains(0)
            for t in range(T):
                main_tile(t)
                if t == 3:
                    emit_transform_mms(1)
                    emit_transform_drains(1)
                if t % 4 == 2 and t > 4:
                    emit_cuboid((t - 6) // 4 if t >= 6 else 0)
            emit_cuboid(7)

            # ---------------- final partial sums ----------------
            out_sb = cp.tile([128, 18], f32)
            scr = cp.tile([128, T * P], f32)
            nc.vector.scalar_tensor_tensor(
                scr, minn.rearrange("n t p -> n (t p)"), 1.0,
                A_all.rearrange("n t p -> n (t p)"), ALU.mult, ALU.mult,
                accum_out=out_sb[:, 0:1])
            nc.vector.scalar_tensor_tensor(
                scr, cub.rearrange("n t p -> n (t p)"), 1.0,
                A_all.rearrange("n t p -> n (t p)"), ALU.mult, ALU.mult,
                accum_out=out_sb[:, 1:2])
            nc.vector.tensor_reduce(out_sb[:, 2:18], A_all.rearrange("n t p -> n p t"),
                                    axis=AX.X, op=ALU.add)
            nc.sync.dma_start(out=out_d.ap(), in_=out_sb)

    nc.compile()
    return nc


def _get_nc():
    if "nc" not in _CACHE:
        _CACHE["nc"] = _build()
    return _CACHE["nc"]


def kernel(**inputs):
    import concourse.bass_utils as bass_utils

    nc = _get_nc()
    names = ["pc", "normals", "trans", "rotate", "scale", "shape_eps",
             "etas", "omegas", "assign_matrix"]
    in_maps = []
    for b in range(B):
        in_maps.append({
            k: np.ascontiguousarray(np.asarray(inputs[k][b], dtype=np.float32))
            for k in names
        })
    res = bass_utils.run_bass_kernel_spmd(nc, in_maps, core_ids=list(range(8)))

    cd_sums, cub_sums, colsums = [], [], []
    for b in range(B):
        o = np.asarray(res.results[b]["out"], dtype=np.float64)
        cd_sums.append(o[:, 0].sum())
        cub_sums.append(o[:, 1].sum())
        colsums.append(o[:, 2:18].sum(axis=0))

    cub = np.sum(cub_sums) / (B * N)
    cd = 2.0 * np.sum(cd_sums) / (B * N)
    ext_terms, sps_terms = [], []
    exist = np.asarray(inputs["exist"], dtype=np.float64)
    for b in range(B):
        gt = (colsums[b] > 24.0).astype(np.float64)
        pr = exist[b, :, 0]
        bce = -(gt * np.maximum(np.log(pr), -100.0)
                + (1 - gt) * np.maximum(np.log(1.0 - pr), -100.0))
        ext_terms.append(bce.mean())
        sps_terms.append(np.sqrt(colsums[b] / N + 0.01).mean() ** 2)
    ext = float(np.mean(ext_terms))
    sps = float(np.mean(sps_terms))
    loss = 1.0 * cub + 1.0 * cd + 0.1 * ext + 0.1 * sps
    return np.float32(loss)


# revision 19
# speedup vs baseline: 1.0031x; 1.0031x over previous
"""Trainium2 Bass kernel for nn_Loss_34230889349355 (superquadric loss).

Data-parallel over B=8 (one batch/core).  Changes vs v1:
- K=5 matmul (lhs rows [pc,1,||pc||^2], rhs rows [-2X',||X'||^2,1]) so PSUM
  holds true squared distances; per-tile relu/bias pass eliminated.
- Balanced ACT/DVE drain, alternating per tile: ACT full-copies 6 (even
  tiles) or 7 (odd tiles) prims/tile to bf16, DVE TR-mins the rest straight
  from PSUM; bf16 folds at DVE 2x; tail TRs batched per 4 tiles.
- Sampling act-ops batched (sin x4 / ln x4 / exp x4) -> 3 act-table loads
  instead of 9.
- Transforms drained by ACT copies into an axis-major bf16 tile; cuboid
  runs on bf16 (DVE 2x / ACT), interleaved with the main loop.
"""

import numpy as np

B, N, P, S = 8, 4096, 16, 200
T = N // 128            # 32 n-tiles
PS = P * S              # 3200 distance columns per n-row

_CACHE = {}


def _build():
    import concourse.bacc as bacc
    import concourse.tile as tile
    import concourse.bass as bass
    from concourse import mybir

    f32 = mybir.dt.float32
    f32r = mybir.dt.float32r
    bf16 = mybir.dt.bfloat16
    ALU = mybir.AluOpType
    ACT = mybir.ActivationFunctionType
    AX = mybir.AxisListType

    nc = bacc.Bacc(
        trn_type="TRN2",
        target_bir_lowering=False,
        debug=False,
        enable_asserts=False,
        num_devices=8,
    )

    pc_d = nc.dram_tensor("pc", [N, 3], f32, kind="ExternalInput")
    nr_d = nc.dram_tensor("normals", [N, 3], f32, kind="ExternalInput")
    tr_d = nc.dram_tensor("trans", [P, 3], f32, kind="ExternalInput")
    ro_d = nc.dram_tensor("rotate", [P, 3, 3], f32, kind="ExternalInput")
    sc_d = nc.dram_tensor("scale", [P, 3], f32, kind="ExternalInput")
    ep_d = nc.dram_tensor("shape_eps", [P, 2], f32, kind="ExternalInput")
    et_d = nc.dram_tensor("etas", [P, S], f32, kind="ExternalInput")
    om_d = nc.dram_tensor("omegas", [P, S], f32, kind="ExternalInput")
    as_d = nc.dram_tensor("assign_matrix", [N, P], f32, kind="ExternalInput")
    out_d = nc.dram_tensor("out", [128, 18], f32, kind="ExternalOutput")

    def dap(tns, ap, offset=0):
        return bass.AP(tensor=tns, offset=offset, ap=ap)

    with tile.TileContext(nc) as tc:
        with (
            tc.tile_pool(name="consts", bufs=1) as cp,
            tc.tile_pool(name="samp", bufs=1) as sp,
            tc.tile_pool(name="work", bufs=4) as wp,
            tc.tile_pool(name="tail", bufs=2) as gp,
            tc.tile_pool(name="psum", bufs=2, space="PSUM") as pp,
        ):
            # ---------------- DMAs (small/critical first) ----------------
            # ones source + const biases (tiny DVE memsets, run at t=0)
            czero = cp.tile([128, 3], f32)
            nc.vector.memset(czero[:, 0:1], 0.0)
            nc.vector.memset(czero[:, 1:2], float(np.pi / 2))
            nc.vector.memset(czero[:, 2:3], 1.0)
            nc.const_aps.aps[(f32, 0.0)] = czero[:, 0:1]
            nc.const_aps.aps[(f32, float(np.pi / 2))] = czero[:, 1:2]

            R4 = cp.tile([4, 3, P], f32r)
            nc.sync.dma_start(out=R4[0:3, :, :],
                              in_=dap(ro_d, [[3, 3], [1, 3], [9, P]]).bitcast(f32r))
            tbT = cp.tile([3, P], f32)
            nc.sync.dma_start(out=tbT, in_=dap(tr_d, [[1, 3], [3, P]]))
            etas = cp.tile([P, S], f32)
            nc.sync.dma_start(out=etas, in_=et_d.ap())
            omegas = cp.tile([P, S], f32)
            nc.sync.dma_start(out=omegas, in_=om_d.ap())
            pc_nt = cp.tile([128, T, 3], f32)
            nc.gpsimd.dma_start(out=pc_nt, in_=dap(pc_d, [[3, 128], [128 * 3, T], [1, 3]]))
            ecols = cp.tile([P, 2], f32)
            nc.sync.dma_start(out=ecols, in_=ep_d.ap())
            acols = cp.tile([P, 3], f32)
            nc.sync.dma_start(out=acols, in_=sc_d.ap())
            tcols = cp.tile([P, 3], f32)
            nc.sync.dma_start(out=tcols, in_=tr_d.ap())
            Rcols = cp.tile([P, 9], f32)
            nc.sync.dma_start(out=Rcols, in_=ro_d.ap().rearrange("p a b -> p (a b)"))
            SCtmp = cp.tile([128, P, 3], f32)
            nc.sync.dma_start(out=SCtmp, in_=dap(sc_d, [[0, 128], [1, P * 3]]))

            pc5T = cp.tile([5, N], f32r)
            nr4T = cp.tile([4, N], f32r)
            for ch in range(4):
                nc.sync.dma_start(
                    out=pc5T[0:3, 1024 * ch: 1024 * (ch + 1)],
                    in_=dap(pc_d, [[1, 3], [3, 1024]], offset=3 * 1024 * ch).bitcast(f32r))
            for ch in range(4):
                nc.sync.dma_start(
                    out=nr4T[0:3, 1024 * ch: 1024 * (ch + 1)],
                    in_=dap(nr_d, [[1, 3], [3, 1024]], offset=3 * 1024 * ch).bitcast(f32r))
            # ones rows via broadcast DMA off czero col 2 (keeps DVE free)
            ones_t = cp.tile([16, 256], f32)
            nc.vector.memset(ones_t, 1.0)
            nc.gpsimd.dma_start(out=pc5T.bitcast(f32)[3:4, :], in_=ones_t)
            nc.gpsimd.dma_start(out=nr4T.bitcast(f32)[3:4, :], in_=ones_t)

            A_all = cp.tile([128, T, P], f32)
            nc.gpsimd.dma_start(out=A_all, in_=dap(as_d, [[P, 128], [128 * P, T], [1, P]]))

            SCf = cp.tile([128, T, 3, P], f32)
            SC = cp.tile([128, T, 3, P], bf16)

            # ||pc_n||^2 -> row 4 of pc5T (col n = t*128 + nr), DRAM roundtrip
            pcsq = cp.tile([128, T, 3], f32)
            nc.vector.tensor_tensor(pcsq, pc_nt, pc_nt, ALU.mult)
            pcn2 = cp.tile([128, T], f32)
            nc.vector.tensor_reduce(pcn2, pcsq, axis=AX.X, op=ALU.add)
            scr_d = nc.dram_tensor("pcn2_scratch", [N], f32, kind="Internal")
            nc.gpsimd.dma_start(out=dap(scr_d, [[1, 128], [128, T]]), in_=pcn2)
            nc.gpsimd.dma_start(out=pc5T.bitcast(f32)[4:5, :],
                              in_=dap(scr_d, [[N, 1], [1, N]]))

            # ---- R4 row 3 + transform matmuls (PE early; drains emitted later) ----
            prod = cp.tile([3, 3, P], f32r)
            for i in range(3):
                nc.vector.tensor_tensor(prod[:, i: i + 1, :], R4[0:3, i: i + 1, :],
                                        tbT.unsqueeze(1), ALU.mult)
            negones3 = cp.tile([3, 1], f32r)
            nc.vector.memset(negones3.bitcast(f32), -1.0)
            rpt = pp.tile([128, 2048], f32, tag="dps", name="rpt")
            nc.tensor.matmul(rpt[0:1, 0:48], negones3,
                             prod.rearrange("a b c -> a (b c)"), start=True, stop=True)
            row3tmp = cp.tile([1, 48], f32r)
            nc.scalar.copy(row3tmp, rpt[0:1, 0:48])
            nc.sync.dma_start(out=R4[3:4, :, :],
                              in_=row3tmp.rearrange("a (i p) -> a i p", i=3))
            pcnI = cp.tile([128, 2, T, 3, P], bf16)   # [:,0]=pcI, [:,1]=nI
            R4f = R4.rearrange("a b c -> a (b c)")
            tf_pts = []

            def emit_transform_mms(half):
                tf = pp.tile([128, 2048], f32, tag="dps", name="tf")
                tfv = tf.rearrange("n (j x) -> n j x", j=16)
                for j in range(16):
                    t = 16 * half + j
                    nc.tensor.matmul(tfv[:, j, 0:48],
                                     pc5T[0:4, 128 * t: 128 * (t + 1)],
                                     R4f, start=True, stop=True)
                    nc.tensor.matmul(tfv[:, j, 64:112],
                                     nr4T[:, 128 * t: 128 * (t + 1)],
                                     R4f, start=True, stop=True)
                tf_pts.append(tfv)

            # ---------------- sampling (batched act tables: 3 loads) ----------
            vals4 = sp.tile([P, 4, S], f32)      # ce, se, co, so
            nc.scalar.activation(vals4[:, 0, :], etas, ACT.Sin, bias=float(np.pi / 2))
            nc.scalar.activation(vals4[:, 1, :], etas, ACT.Sin)
            nc.scalar.activation(vals4[:, 2, :], omegas, ACT.Sin, bias=float(np.pi / 2))
            nc.scalar.activation(vals4[:, 3, :], omegas, ACT.Sin)
            av4 = sp.tile([P, 4, S], f32)
            nc.scalar.activation(av4, vals4, ACT.Abs)
            nc.scalar.activation(av4, av4, ACT.Ln)
            e1 = ecols[:, 0:1]
            e2 = ecols[:, 1:2]
            nc.vector.tensor_scalar(av4[:, 0:2, :], av4[:, 0:2, :], e1, None, ALU.mult)
            nc.vector.tensor_scalar(av4[:, 2:4, :], av4[:, 2:4, :], e2, None, ALU.mult)
            nc.scalar.activation(av4, av4, ACT.Exp)
            sg4 = sp.tile([P, 4, S], f32)
            nc.scalar.activation(sg4, vals4, ACT.Sign)

            def clampc(mi1, mi2, a_col, nm):
                # sign(v1)sign(v2) * max(a*|f1|*|f2|, 1e-6); av4 holds |f|^e
                m = sp.tile([P, S], f32, tag=nm + "_m", name=nm + "_m")
                if mi2 is not None:
                    nc.vector.tensor_tensor(m, av4[:, mi1, :], av4[:, mi2, :], ALU.mult)
                    nc.vector.tensor_scalar(m, m, a_col, None, ALU.mult)
                else:
                    nc.vector.tensor_scalar(m, av4[:, mi1, :], a_col, None, ALU.mult)
                nc.vector.tensor_scalar(m, m, 1e-6, None, ALU.max)
                if mi2 is not None:
                    s = sp.tile([P, S], f32, tag=nm + "_s", name=nm + "_s")
                    nc.vector.tensor_tensor(s, sg4[:, mi1, :], sg4[:, mi2, :], ALU.mult)
                    nc.vector.tensor_tensor(m, m, s, ALU.mult)
                else:
                    nc.vector.tensor_tensor(m, m, sg4[:, mi1, :], ALU.mult)
                return m

            xc = clampc(0, 2, acols[:, 0:1], "xc")
            yc = clampc(0, 3, acols[:, 1:2], "yc")
            zc = clampc(1, None, acols[:, 2:3], "zc")

            # world frame: X'' = -2(R X + t); rhs5 rows 0-2 = X''_i,
            # row 3 = ||X''||^2/4 (= ||X'||^2), row 4 = ones
            R2 = sp.tile([P, 9], f32)
            nc.vector.tensor_scalar(R2, Rcols, -2.0, None, ALU.mult)
            t2 = sp.tile([P, 3], f32)
            nc.vector.tensor_scalar(t2, tcols, -2.0, None, ALU.mult)

            rhs5 = cp.tile([5, PS], f32r)
            nc.gpsimd.dma_start(out=rhs5.bitcast(f32)[4:5, :], in_=ones_t[:, 0:200])
            Xp = []
            for i in range(3):
                u = sp.tile([P, S], f32r, tag=f"xp{i}", name=f"xp{i}")
                nc.vector.tensor_scalar(u, xc, R2[:, 3 * i + 0: 3 * i + 1], None, ALU.mult)
                nc.vector.scalar_tensor_tensor(u, yc, R2[:, 3 * i + 1: 3 * i + 2], u,
                                               ALU.mult, ALU.add)
                nc.vector.scalar_tensor_tensor(u, zc, R2[:, 3 * i + 2: 3 * i + 3], u,
                                               ALU.mult, ALU.add)
                nc.vector.tensor_scalar(u, u, t2[:, i: i + 1], None, ALU.add)
                Xp.append(u)
            sq0 = sp.tile([P, S], f32r)
            nc.vector.tensor_tensor(sq0, Xp[0], Xp[0], ALU.mult)
            sq1 = sp.tile([P, S], f32r)
            nc.vector.tensor_tensor(sq1, Xp[1], Xp[1], ALU.mult)
            nc.vector.tensor_tensor(sq0, sq0, sq1, ALU.add)
            nc.vector.tensor_tensor(sq1, Xp[2], Xp[2], ALU.mult)
            nc.vector.tensor_tensor(sq0, sq0, sq1, ALU.add)
            nc.vector.tensor_scalar(sq0, sq0, 0.25, None, ALU.mult)
            for i, src_t in enumerate(Xp + [sq0]):
                nc.gpsimd.dma_start(
                    out=rhs5[i: i + 1, :].rearrange("a (p s) -> a p s", p=P),
                    in_=src_t)

            nc.vector.tensor_copy(SCf[:, 0, :, :], SCtmp.rearrange("n p i -> n i p"))
            _w = 1
            while _w < T:
                _c = min(_w, T - _w)
                nc.vector.tensor_copy(SCf[:, _w:_w + _c, :, :], SCf[:, 0:_c, :, :])
                _w += _c
            nc.vector.tensor_copy(SC, SCf)

            def emit_transform_drains(half):
                tfv = tf_pts[half]
                nc.scalar.copy(
                    pcnI[:, 0, 16 * half: 16 * half + 16, :, :],
                    tfv[:, :, 0:48].rearrange("n t (i p) -> n t i p", i=3))
                nc.scalar.copy(
                    pcnI[:, 1, 16 * half: 16 * half + 16, :, :],
                    tfv[:, :, 64:112].rearrange("n t (i p) -> n t i p", i=3))

            # ---------------- cuboid (batched bf16, interleaved) --------------
            pcI = pcnI[:, 0]   # [128, T, 3, P]
            nI = pcnI[:, 1]
            cub = cp.tile([128, T, P], f32)
            cbt = {}

            def cb(nm, shape=None, dtype=bf16):
                if nm not in cbt:
                    cbt[nm] = cp.tile(shape or [128, T, 3, P], dtype,
                                      tag="cb_" + nm, name="cb_" + nm)
                return cbt[nm]

            def emit_cuboid(step):
                if step == 0:
                    ax = cb("ax")
                    nc.scalar.activation(ax, pcI, ACT.Abs)
                    w1 = cb("w1")
                    nc.vector.tensor_tensor(w1, ax, SC, ALU.subtract)
                elif step == 1:
                    w1 = cbt["w1"]
                    nc.scalar.activation(w1, w1, ACT.Relu)
                    ee = cb("ee")
                    nc.scalar.activation(ee, w1, ACT.Square)
                elif step == 2:
                    gg = cb("gg")
                    nc.vector.tensor_scalar(gg, nI, 0.0, None, ALU.is_gt)
                    mm_ = cb("mm")
                    nc.vector.tensor_tensor(mm_, gg, pcI, ALU.mult)
                elif step == 3:
                    u = cb("u")
                    nc.vector.scalar_tensor_tensor(u, cbt["mm"], 2.0, pcI, ALU.mult,
                                                   ALU.subtract)
                    nc.vector.tensor_tensor(u, u, SC, ALU.subtract)
                elif step == 4:
                    qq = cb("qq")
                    nc.scalar.activation(qq, cbt["u"], ACT.Square)
                    dd = cb("dd")
                    nc.vector.tensor_tensor(dd, qq, cbt["ee"], ALU.subtract)
                elif step == 5:
                    tA = cb("tA")
                    nc.scalar.activation(tA, nI, ACT.Abs)
                    E = cb("E", [128, T, P])
                    nc.vector.tensor_tensor(E, cbt["ee"][:, :, 0, :],
                                            cbt["ee"][:, :, 1, :], ALU.add)
                    nc.vector.tensor_tensor(E, E, cbt["ee"][:, :, 2, :], ALU.add)
                elif step == 6:
                    tA = cbt["tA"]
                    c1 = cb("c1", [128, T, P], mybir.dt.uint8)
                    nc.vector.tensor_tensor(c1, tA[:, :, 0, :], tA[:, :, 1, :], ALU.is_ge)
                    t1 = cb("t1", [128, T, P])
                    nc.vector.tensor_tensor(t1, tA[:, :, 0, :], tA[:, :, 1, :], ALU.max)
                    c2 = cb("c2", [128, T, P], mybir.dt.uint8)
                    nc.vector.tensor_tensor(c2, t1, tA[:, :, 2, :], ALU.is_ge)
                elif step == 7:
                    dd = cbt["dd"]
                    d1 = cb("d1", [128, T, P])
                    nc.vector.select(d1, cbt["c1"], dd[:, :, 0, :], dd[:, :, 1, :])
                    dsel = cb("dsel", [128, T, P])
                    nc.vector.select(dsel, cbt["c2"], d1, dd[:, :, 2, :])
                    nc.vector.tensor_tensor(cub, cbt["E"], dsel, ALU.add)

            # ---------------- main loop ----------------
            minn = cp.tile([128, T, P], f32)
            G4_cur = [None]

            state = {}

            def emit_folds(t):
                even = (t % 2 == 0)
                sbA, F1, F2, G4 = state[t]
                if even:
                    nc.vector.tensor_tensor(
                        F1[:, :, 0:6, :], sbA[:, :, 0:6, 0:100],
                        sbA[:, :, 0:6, 100:200], ALU.min)
                    nc.vector.tensor_tensor(F2[:, :, 0:6, :], F1[:, :, 0:6, 0:50],
                                            F1[:, :, 0:6, 50:100], ALU.min)
                else:
                    nc.vector.tensor_tensor(F1[:, 0, :, :], sbA[:, 0, :, 0:100],
                                            sbA[:, 0, :, 100:200], ALU.min)
                    nc.vector.tensor_tensor(F1[:, 1, 0:6, :], sbA[:, 1, 0:6, 0:100],
                                            sbA[:, 1, 0:6, 100:200], ALU.min)
                    nc.vector.tensor_tensor(F2, F1[:, :, :, 0:50], F1[:, :, :, 50:100],
                                            ALU.min)
                nc.vector.tensor_tensor(G4[:, t % 4], F2[:, :, :, 0:25],
                                        F2[:, :, :, 25:50], ALU.min)
                del state[t]
                if t % 4 == 3:
                    g = t // 4
                    nc.vector.tensor_reduce(
                        minn[:, 4 * g: 4 * g + 4: 2, :]
                            .rearrange("n t (h p) -> n t h p", h=2)[:, :, :, 0:6],
                        G4[:, 0:4:2, :, 0:6, :], axis=AX.X, op=ALU.min)
                    nc.vector.tensor_reduce(
                        minn[:, 4 * g + 1: 4 * g + 4: 2, 0:8],
                        G4[:, 1:4:2, 0, :, :], axis=AX.X, op=ALU.min)
                    nc.vector.tensor_reduce(
                        minn[:, 4 * g + 1: 4 * g + 4: 2, 8:14],
                        G4[:, 1:4:2, 1, 0:6, :], axis=AX.X, op=ALU.min)

            def main_tile(t):
                even = (t % 2 == 0)
                sbA = wp.tile([128, 2, 8, 200], bf16, tag="sbA", name="sbA")
                F1 = wp.tile([128, 2, 8, 100], bf16, tag="F1", name="F1")
                F2 = wp.tile([128, 2, 8, 50], bf16, tag="F2", name="F2")
                if t % 4 == 0:
                    G4_cur[0] = gp.tile([128, 4, 2, 8, 25], bf16, tag="G4", name="G4")
                state[t] = (sbA, F1, F2, G4_cur[0])
                for h in range(2):
                    dt = pp.tile([128, 2048], f32, tag="dps", name="dt")
                    dv = dt.rearrange("n (a x) -> n a x", a=4)
                    for q in range(4):
                        nc.tensor.matmul(
                            dv[:, q, 0:400], pc5T[:, 128 * t: 128 * (t + 1)],
                            rhs5[:, 1600 * h + 400 * q: 1600 * h + 400 * (q + 1)],
                            start=True, stop=True)
                    nacts = 4 if (not even and h == 0) else 3
                    k = 2 * nacts
                    nc.scalar.copy(
                        sbA[:, h, 0:k, :].rearrange("n (b p) s -> n b p s", b=nacts),
                        dv[:, 0:nacts, 0:400].rearrange("n b (p s) -> n b p s", p=2))
                    if nacts == 3:
                        nc.vector.tensor_reduce(
                            minn[:, t, 8 * h + 6: 8 * h + 8],
                            dv[:, 3, 0:400].rearrange("n (p s) -> n p s", p=2),
                            axis=AX.X, op=ALU.min)
                if t > 0:
                    emit_folds(t - 1)

            emit_transform_mms(0)
            emit_transform_drains(0)
            for t in range(T):
                main_tile(t)
                if t == 3:
                    emit_transform_mms(1)
                    emit_transform_drains(1)
                if t % 4 == 2 and t > 4:
                    emit_cuboid((t - 6) // 4 if t >= 6 else 0)
            emit_cuboid(7)
            emit_folds(T - 1)

            # ---------------- final partial sums ----------------
            out_sb = cp.tile([128, 18], f32)
            scr = cp.tile([128, T * P], f32)
            nc.vector.scalar_tensor_tensor(
                scr, minn.rearrange("n t p -> n (t p)"), 1.0,
                A_all.rearrange("n t p -> n (t p)"), ALU.mult, ALU.mult,
                accum_out=out_sb[:, 0:1])
            nc.vector.scalar_tensor_tensor(
                scr, cub.rearrange("n t p -> n (t p)"), 1.0,
                A_all.rearrange("n t p -> n (t p)"), ALU.mult, ALU.mult,
                accum_out=out_sb[:, 1:2])
            nc.vector.tensor_reduce(out_sb[:, 2:18], A_all.rearrange("n t p -> n p t"),
                                    axis=AX.X, op=ALU.add)
            nc.sync.dma_start(out=out_d.ap(), in_=out_sb)

    nc.compile()
    return nc


def _get_nc():
    if "nc" not in _CACHE:
        _CACHE["nc"] = _build()
    return _CACHE["nc"]


def kernel(**inputs):
    import concourse.bass_utils as bass_utils

    nc = _get_nc()
    names = ["pc", "normals", "trans", "rotate", "scale", "shape_eps",
             "etas", "omegas", "assign_matrix"]
    in_maps = []
    for b in range(B):
        in_maps.append({
            k: np.ascontiguousarray(np.asarray(inputs[k][b], dtype=np.float32))
            for k in names
        })
    res = bass_utils.run_bass_kernel_spmd(nc, in_maps, core_ids=list(range(8)))

    cd_sums, cub_sums, colsums = [], [], []
    for b in range(B):
        o = np.asarray(res.results[b]["out"], dtype=np.float64)
        cd_sums.append(o[:, 0].sum())
        cub_sums.append(o[:, 1].sum())
        colsums.append(o[:, 2:18].sum(axis=0))

    cub = np.sum(cub_sums) / (B * N)
    cd = 2.0 * np.sum(cd_sums) / (B * N)
    ext_terms, sps_terms = [], []
    exist = np.asarray(inputs["exist"], dtype=np.float64)
    for b in range(B):
        gt = (colsums[b] > 24.0).astype(np.float64)
        pr = exist[b, :, 0]
        bce = -(gt * np.maximum(np.log(pr), -100.0)
                + (1 - gt) * np.maximum(np.log(1.0 - pr), -100.0))
        ext_terms.append(bce.mean())
        sps_terms.append(np.sqrt(colsums[b] / N + 0.01).mean() ** 2)
    ext = float(np.mean(ext_terms))
    sps = float(np.mean(sps_terms))
    loss = 1.0 * cub + 1.0 * cd + 0.1 * ext + 0.1 * sps
    return np.float32(loss)


# revision 20
# speedup vs baseline: 1.0032x; 1.0001x over previous
"""Trainium2 Bass kernel for nn_Loss_34230889349355 (superquadric loss).

Data-parallel over B=8 (one batch/core).  Changes vs v1:
- K=5 matmul (lhs rows [pc,1,||pc||^2], rhs rows [-2X',||X'||^2,1]) so PSUM
  holds true squared distances; per-tile relu/bias pass eliminated.
- Balanced ACT/DVE drain, alternating per tile: ACT full-copies 6 (even
  tiles) or 7 (odd tiles) prims/tile to bf16, DVE TR-mins the rest straight
  from PSUM; bf16 folds at DVE 2x; tail TRs batched per 4 tiles.
- Sampling act-ops batched (sin x4 / ln x4 / exp x4) -> 3 act-table loads
  instead of 9.
- Transforms drained by ACT copies into an axis-major bf16 tile; cuboid
  runs on bf16 (DVE 2x / ACT), interleaved with the main loop.
"""

import numpy as np

B, N, P, S = 8, 4096, 16, 200
T = N // 128            # 32 n-tiles
PS = P * S              # 3200 distance columns per n-row

_CACHE = {}


def _build():
    import concourse.bacc as bacc
    import concourse.tile as tile
    import concourse.bass as bass
    from concourse import mybir

    f32 = mybir.dt.float32
    f32r = mybir.dt.float32r
    bf16 = mybir.dt.bfloat16
    ALU = mybir.AluOpType
    ACT = mybir.ActivationFunctionType
    AX = mybir.AxisListType

    nc = bacc.Bacc(
        trn_type="TRN2",
        target_bir_lowering=False,
        debug=False,
        enable_asserts=False,
        num_devices=8,
    )

    pc_d = nc.dram_tensor("pc", [N, 3], f32, kind="ExternalInput")
    nr_d = nc.dram_tensor("normals", [N, 3], f32, kind="ExternalInput")
    tr_d = nc.dram_tensor("trans", [P, 3], f32, kind="ExternalInput")
    ro_d = nc.dram_tensor("rotate", [P, 3, 3], f32, kind="ExternalInput")
    sc_d = nc.dram_tensor("scale", [P, 3], f32, kind="ExternalInput")
    ep_d = nc.dram_tensor("shape_eps", [P, 2], f32, kind="ExternalInput")
    et_d = nc.dram_tensor("etas", [P, S], f32, kind="ExternalInput")
    om_d = nc.dram_tensor("omegas", [P, S], f32, kind="ExternalInput")
    as_d = nc.dram_tensor("assign_matrix", [N, P], f32, kind="ExternalInput")
    out_d = nc.dram_tensor("out", [128, 18], f32, kind="ExternalOutput")

    def dap(tns, ap, offset=0):
        return bass.AP(tensor=tns, offset=offset, ap=ap)

    with tile.TileContext(nc) as tc:
        with (
            tc.tile_pool(name="consts", bufs=1) as cp,
            tc.tile_pool(name="samp", bufs=1) as sp,
            tc.tile_pool(name="work", bufs=4) as wp,
            tc.tile_pool(name="tail", bufs=2) as gp,
            tc.tile_pool(name="psum", bufs=2, space="PSUM") as pp,
        ):
            # ---------------- DMAs (small/critical first) ----------------
            # ones source + const biases (tiny DVE memsets, run at t=0)
            czero = cp.tile([128, 3], f32)
            nc.vector.memset(czero[:, 0:1], 0.0)
            nc.vector.memset(czero[:, 1:2], float(np.pi / 2))
            nc.vector.memset(czero[:, 2:3], 1.0)
            nc.const_aps.aps[(f32, 0.0)] = czero[:, 0:1]
            nc.const_aps.aps[(f32, float(np.pi / 2))] = czero[:, 1:2]

            etas = cp.tile([P, S], f32)
            nc.sync.dma_start(out=etas, in_=et_d.ap())
            omegas = cp.tile([P, S], f32)
            nc.sync.dma_start(out=omegas, in_=om_d.ap())
            R4 = cp.tile([4, 3, P], f32r)
            nc.sync.dma_start(out=R4[0:3, :, :],
                              in_=dap(ro_d, [[3, 3], [1, 3], [9, P]]).bitcast(f32r))
            tbT = cp.tile([3, P], f32)
            nc.sync.dma_start(out=tbT, in_=dap(tr_d, [[1, 3], [3, P]]))
            pc_nt = cp.tile([128, T, 3], f32)
            nc.gpsimd.dma_start(out=pc_nt, in_=dap(pc_d, [[3, 128], [128 * 3, T], [1, 3]]))
            ecols = cp.tile([P, 2], f32)
            nc.sync.dma_start(out=ecols, in_=ep_d.ap())
            acols = cp.tile([P, 3], f32)
            nc.sync.dma_start(out=acols, in_=sc_d.ap())
            tcols = cp.tile([P, 3], f32)
            nc.sync.dma_start(out=tcols, in_=tr_d.ap())
            Rcols = cp.tile([P, 9], f32)
            nc.sync.dma_start(out=Rcols, in_=ro_d.ap().rearrange("p a b -> p (a b)"))
            SCtmp = cp.tile([128, P, 3], f32)
            nc.sync.dma_start(out=SCtmp, in_=dap(sc_d, [[0, 128], [1, P * 3]]))

            pc5T = cp.tile([5, N], f32r)
            nr4T = cp.tile([4, N], f32r)
            for ch in range(4):
                nc.sync.dma_start(
                    out=pc5T[0:3, 1024 * ch: 1024 * (ch + 1)],
                    in_=dap(pc_d, [[1, 3], [3, 1024]], offset=3 * 1024 * ch).bitcast(f32r))
            for ch in range(4):
                nc.sync.dma_start(
                    out=nr4T[0:3, 1024 * ch: 1024 * (ch + 1)],
                    in_=dap(nr_d, [[1, 3], [3, 1024]], offset=3 * 1024 * ch).bitcast(f32r))
            # ones rows via broadcast DMA off czero col 2 (keeps DVE free)
            ones_t = cp.tile([16, 256], f32)
            nc.vector.memset(ones_t, 1.0)
            nc.gpsimd.dma_start(out=pc5T.bitcast(f32)[3:4, :], in_=ones_t)
            nc.gpsimd.dma_start(out=nr4T.bitcast(f32)[3:4, :], in_=ones_t)

            A_all = cp.tile([128, T, P], f32)
            nc.gpsimd.dma_start(out=A_all, in_=dap(as_d, [[P, 128], [128 * P, T], [1, P]]))

            SCf = cp.tile([128, T, 3, P], f32)
            SC = cp.tile([128, T, 3, P], bf16)

            # ||pc_n||^2 -> row 4 of pc5T (col n = t*128 + nr), DRAM roundtrip
            pcsq = cp.tile([128, T, 3], f32)
            nc.vector.tensor_tensor(pcsq, pc_nt, pc_nt, ALU.mult)
            pcn2 = cp.tile([128, T], f32)
            nc.vector.tensor_reduce(pcn2, pcsq, axis=AX.X, op=ALU.add)
            scr_d = nc.dram_tensor("pcn2_scratch", [N], f32, kind="Internal")
            nc.gpsimd.dma_start(out=dap(scr_d, [[1, 128], [128, T]]), in_=pcn2)
            nc.gpsimd.dma_start(out=pc5T.bitcast(f32)[4:5, :],
                              in_=dap(scr_d, [[N, 1], [1, N]]))

            # ---- R4 row 3 + transform matmuls (PE early; drains emitted later) ----
            prod = cp.tile([3, 3, P], f32r)
            for i in range(3):
                nc.vector.tensor_tensor(prod[:, i: i + 1, :], R4[0:3, i: i + 1, :],
                                        tbT.unsqueeze(1), ALU.mult)
            negones3 = cp.tile([3, 1], f32r)
            nc.vector.memset(negones3.bitcast(f32), -1.0)
            rpt = pp.tile([128, 2048], f32, tag="dps", name="rpt")
            nc.tensor.matmul(rpt[0:1, 0:48], negones3,
                             prod.rearrange("a b c -> a (b c)"), start=True, stop=True)
            row3tmp = cp.tile([1, 48], f32r)
            nc.scalar.copy(row3tmp, rpt[0:1, 0:48])
            nc.sync.dma_start(out=R4[3:4, :, :],
                              in_=row3tmp.rearrange("a (i p) -> a i p", i=3))
            pcnI = cp.tile([128, 2, T, 3, P], bf16)   # [:,0]=pcI, [:,1]=nI
            R4f = R4.rearrange("a b c -> a (b c)")
            tf_pts = []

            def emit_transform_mms(half):
                tf = pp.tile([128, 2048], f32, tag="dps", name="tf")
                tfv = tf.rearrange("n (j x) -> n j x", j=16)
                for j in range(16):
                    t = 16 * half + j
                    nc.tensor.matmul(tfv[:, j, 0:48],
                                     pc5T[0:4, 128 * t: 128 * (t + 1)],
                                     R4f, start=True, stop=True)
                    nc.tensor.matmul(tfv[:, j, 64:112],
                                     nr4T[:, 128 * t: 128 * (t + 1)],
                                     R4f, start=True, stop=True)
                tf_pts.append(tfv)

            # ---------------- sampling (batched act tables: 3 loads) ----------
            vals4 = sp.tile([P, 4, S], f32)      # ce, se, co, so
            nc.scalar.activation(vals4[:, 0, :], etas, ACT.Sin, bias=float(np.pi / 2))
            nc.scalar.activation(vals4[:, 1, :], etas, ACT.Sin)
            nc.scalar.activation(vals4[:, 2, :], omegas, ACT.Sin, bias=float(np.pi / 2))
            nc.scalar.activation(vals4[:, 3, :], omegas, ACT.Sin)
            av4 = sp.tile([P, 4, S], f32)
            nc.scalar.activation(av4, vals4, ACT.Abs)
            nc.scalar.activation(av4, av4, ACT.Ln)
            e1 = ecols[:, 0:1]
            e2 = ecols[:, 1:2]
            nc.vector.tensor_scalar(av4[:, 0:2, :], av4[:, 0:2, :], e1, None, ALU.mult)
            nc.vector.tensor_scalar(av4[:, 2:4, :], av4[:, 2:4, :], e2, None, ALU.mult)
            nc.scalar.activation(av4, av4, ACT.Exp)
            sg4 = sp.tile([P, 4, S], f32)
            nc.scalar.activation(sg4, vals4, ACT.Sign)

            def clampc(mi1, mi2, a_col, nm):
                # sign(v1)sign(v2) * max(a*|f1|*|f2|, 1e-6); av4 holds |f|^e
                m = sp.tile([P, S], f32, tag=nm + "_m", name=nm + "_m")
                if mi2 is not None:
                    nc.vector.tensor_tensor(m, av4[:, mi1, :], av4[:, mi2, :], ALU.mult)
                    nc.vector.tensor_scalar(m, m, a_col, None, ALU.mult)
                else:
                    nc.vector.tensor_scalar(m, av4[:, mi1, :], a_col, None, ALU.mult)
                nc.vector.tensor_scalar(m, m, 1e-6, None, ALU.max)
                if mi2 is not None:
                    s = sp.tile([P, S], f32, tag=nm + "_s", name=nm + "_s")
                    nc.vector.tensor_tensor(s, sg4[:, mi1, :], sg4[:, mi2, :], ALU.mult)
                    nc.vector.tensor_tensor(m, m, s, ALU.mult)
                else:
                    nc.vector.tensor_tensor(m, m, sg4[:, mi1, :], ALU.mult)
                return m

            xc = clampc(0, 2, acols[:, 0:1], "xc")
            yc = clampc(0, 3, acols[:, 1:2], "yc")
            zc = clampc(1, None, acols[:, 2:3], "zc")

            # world frame: X'' = -2(R X + t); rhs5 rows 0-2 = X''_i,
            # row 3 = ||X''||^2/4 (= ||X'||^2), row 4 = ones
            R2 = sp.tile([P, 9], f32)
            nc.vector.tensor_scalar(R2, Rcols, -2.0, None, ALU.mult)
            t2 = sp.tile([P, 3], f32)
            nc.vector.tensor_scalar(t2, tcols, -2.0, None, ALU.mult)

            rhs5 = cp.tile([5, PS], f32r)
            nc.gpsimd.dma_start(out=rhs5.bitcast(f32)[4:5, :], in_=ones_t[:, 0:200])
            Xp = []
            sq0 = sp.tile([P, S], f32r)
            sq1 = sp.tile([P, S], f32r)
            for i in range(3):
                u = sp.tile([P, S], f32r, tag=f"xp{i}", name=f"xp{i}")
                nc.vector.tensor_scalar(u, xc, R2[:, 3 * i + 0: 3 * i + 1], None, ALU.mult)
                nc.vector.scalar_tensor_tensor(u, yc, R2[:, 3 * i + 1: 3 * i + 2], u,
                                               ALU.mult, ALU.add)
                nc.vector.scalar_tensor_tensor(u, zc, R2[:, 3 * i + 2: 3 * i + 3], u,
                                               ALU.mult, ALU.add)
                nc.vector.tensor_scalar(u, u, t2[:, i: i + 1], None, ALU.add)
                Xp.append(u)
                if i == 0:
                    nc.vector.tensor_tensor(sq0, u, u, ALU.mult)
                else:
                    nc.vector.tensor_tensor(sq1, u, u, ALU.mult)
                    nc.vector.tensor_tensor(sq0, sq0, sq1, ALU.add)
            nc.vector.tensor_scalar(sq0, sq0, 0.25, None, ALU.mult)
            for i, src_t in enumerate(Xp + [sq0]):
                eng = nc.sync if i % 2 == 0 else nc.gpsimd
                eng.dma_start(
                    out=rhs5[i: i + 1, :].rearrange("a (p s) -> a p s", p=P),
                    in_=src_t)

            nc.vector.tensor_copy(SCf[:, 0, :, :], SCtmp.rearrange("n p i -> n i p"))
            _w = 1
            while _w < T:
                _c = min(_w, T - _w)
                nc.vector.tensor_copy(SCf[:, _w:_w + _c, :, :], SCf[:, 0:_c, :, :])
                _w += _c
            nc.vector.tensor_copy(SC, SCf)

            def emit_transform_drains(half):
                tfv = tf_pts[half]
                nc.scalar.copy(
                    pcnI[:, 0, 16 * half: 16 * half + 16, :, :],
                    tfv[:, :, 0:48].rearrange("n t (i p) -> n t i p", i=3))
                nc.scalar.copy(
                    pcnI[:, 1, 16 * half: 16 * half + 16, :, :],
                    tfv[:, :, 64:112].rearrange("n t (i p) -> n t i p", i=3))

            # ---------------- cuboid (batched bf16, interleaved) --------------
            pcI = pcnI[:, 0]   # [128, T, 3, P]
            nI = pcnI[:, 1]
            cub = cp.tile([128, T, P], f32)
            cbt = {}

            def cb(nm, shape=None, dtype=bf16):
                if nm not in cbt:
                    cbt[nm] = cp.tile(shape or [128, T, 3, P], dtype,
                                      tag="cb_" + nm, name="cb_" + nm)
                return cbt[nm]

            def emit_cuboid(step):
                if step == 0:
                    ax = cb("ax")
                    nc.scalar.activation(ax, pcI, ACT.Abs)
                    w1 = cb("w1")
                    nc.vector.tensor_tensor(w1, ax, SC, ALU.subtract)
                elif step == 1:
                    w1 = cbt["w1"]
                    nc.scalar.activation(w1, w1, ACT.Relu)
                    ee = cb("ee")
                    nc.scalar.activation(ee, w1, ACT.Square)
                elif step == 2:
                    gg = cb("gg")
                    nc.vector.tensor_scalar(gg, nI, 0.0, None, ALU.is_gt)
                    mm_ = cb("mm")
                    nc.vector.tensor_tensor(mm_, gg, pcI, ALU.mult)
                elif step == 3:
                    u = cb("u")
                    nc.vector.scalar_tensor_tensor(u, cbt["mm"], 2.0, pcI, ALU.mult,
                                                   ALU.subtract)
                    nc.vector.tensor_tensor(u, u, SC, ALU.subtract)
                elif step == 4:
                    qq = cb("qq")
                    nc.scalar.activation(qq, cbt["u"], ACT.Square)
                    dd = cb("dd")
                    nc.vector.tensor_tensor(dd, qq, cbt["ee"], ALU.subtract)
                elif step == 5:
                    tA = cb("tA")
                    nc.scalar.activation(tA, nI, ACT.Abs)
                    E = cb("E", [128, T, P])
                    nc.vector.tensor_tensor(E, cbt["ee"][:, :, 0, :],
                                            cbt["ee"][:, :, 1, :], ALU.add)
                    nc.vector.tensor_tensor(E, E, cbt["ee"][:, :, 2, :], ALU.add)
                elif step == 6:
                    tA = cbt["tA"]
                    c1 = cb("c1", [128, T, P], mybir.dt.uint8)
                    nc.vector.tensor_tensor(c1, tA[:, :, 0, :], tA[:, :, 1, :], ALU.is_ge)
                    t1 = cb("t1", [128, T, P])
                    nc.vector.tensor_tensor(t1, tA[:, :, 0, :], tA[:, :, 1, :], ALU.max)
                    c2 = cb("c2", [128, T, P], mybir.dt.uint8)
                    nc.vector.tensor_tensor(c2, t1, tA[:, :, 2, :], ALU.is_ge)
                elif step == 7:
                    dd = cbt["dd"]
                    d1 = cb("d1", [128, T, P])
                    nc.vector.select(d1, cbt["c1"], dd[:, :, 0, :], dd[:, :, 1, :])
                    dsel = cb("dsel", [128, T, P])
                    nc.vector.select(dsel, cbt["c2"], d1, dd[:, :, 2, :])
                    nc.vector.tensor_tensor(cub, cbt["E"], dsel, ALU.add)

            # ---------------- main loop ----------------
            minn = cp.tile([128, T, P], f32)
            G4_cur = [None]

            state = {}

            def emit_folds(t):
                even = (t % 2 == 0)
                sbA, F1, F2, G4 = state[t]
                if even:
                    nc.vector.tensor_tensor(
                        F1[:, :, 0:6, :], sbA[:, :, 0:6, 0:100],
                        sbA[:, :, 0:6, 100:200], ALU.min)
                    nc.vector.tensor_tensor(F2[:, :, 0:6, :], F1[:, :, 0:6, 0:50],
                                            F1[:, :, 0:6, 50:100], ALU.min)
                else:
                    nc.vector.tensor_tensor(F1[:, 0, :, :], sbA[:, 0, :, 0:100],
                                            sbA[:, 0, :, 100:200], ALU.min)
                    nc.vector.tensor_tensor(F1[:, 1, 0:6, :], sbA[:, 1, 0:6, 0:100],
                                            sbA[:, 1, 0:6, 100:200], ALU.min)
                    nc.vector.tensor_tensor(F2, F1[:, :, :, 0:50], F1[:, :, :, 50:100],
                                            ALU.min)
                nc.vector.tensor_tensor(G4[:, t % 4], F2[:, :, :, 0:25],
                                        F2[:, :, :, 25:50], ALU.min)
                del state[t]
                if t % 4 == 3:
                    g = t // 4
                    nc.vector.tensor_reduce(
                        minn[:, 4 * g: 4 * g + 4: 2, :]
                            .rearrange("n t (h p) -> n t h p", h=2)[:, :, :, 0:6],
                        G4[:, 0:4:2, :, 0:6, :], axis=AX.X, op=ALU.min)
                    nc.vector.tensor_reduce(
                        minn[:, 4 * g + 1: 4 * g + 4: 2, 0:8],
                        G4[:, 1:4:2, 0, :, :], axis=AX.X, op=ALU.min)
                    nc.vector.tensor_reduce(
                        minn[:, 4 * g + 1: 4 * g + 4: 2, 8:14],
                        G4[:, 1:4:2, 1, 0:6, :], axis=AX.X, op=ALU.min)

            def main_tile(t):
                even = (t % 2 == 0)
                sbA = wp.tile([128, 2, 8, 200], bf16, tag="sbA", name="sbA")
                F1 = wp.tile([128, 2, 8, 100], bf16, tag="F1", name="F1")
                F2 = wp.tile([128, 2, 8, 50], bf16, tag="F2", name="F2")
                if t % 4 == 0:
                    G4_cur[0] = gp.tile([128, 4, 2, 8, 25], bf16, tag="G4", name="G4")
                state[t] = (sbA, F1, F2, G4_cur[0])
                for h in range(2):
                    dt = pp.tile([128, 2048], f32, tag="dps", name="dt")
                    dv = dt.rearrange("n (a x) -> n a x", a=4)
                    for q in range(4):
                        nc.tensor.matmul(
                            dv[:, q, 0:400], pc5T[:, 128 * t: 128 * (t + 1)],
                            rhs5[:, 1600 * h + 400 * q: 1600 * h + 400 * (q + 1)],
                            start=True, stop=True)
                    nacts = 4 if (not even and h == 0) else 3
                    k = 2 * nacts
                    nc.scalar.copy(
                        sbA[:, h, 0:k, :].rearrange("n (b p) s -> n b p s", b=nacts),
                        dv[:, 0:nacts, 0:400].rearrange("n b (p s) -> n b p s", p=2))
                    if nacts == 3:
                        nc.vector.tensor_reduce(
                            minn[:, t, 8 * h + 6: 8 * h + 8],
                            dv[:, 3, 0:400].rearrange("n (p s) -> n p s", p=2),
                            axis=AX.X, op=ALU.min)
                if t > 0:
                    emit_folds(t - 1)

            emit_transform_mms(0)
            emit_transform_drains(0)
            for t in range(T):
                main_tile(t)
                if t == 3:
                    emit_transform_mms(1)
                    emit_transform_drains(1)
                if t % 4 == 2 and t > 4:
                    emit_cuboid((t - 6) // 4 if t >= 6 else 0)
            emit_cuboid(7)
            emit_folds(T - 1)

            # ---------------- final partial sums ----------------
            out_sb = cp.tile([128, 18], f32)
            scr = cp.tile([128, T * P], f32)
            nc.vector.scalar_tensor_tensor(
                scr, minn.rearrange("n t p -> n (t p)"), 1.0,
                A_all.rearrange("n t p -> n (t p)"), ALU.mult, ALU.mult,
                accum_out=out_sb[:, 0:1])
            nc.vector.scalar_tensor_tensor(
                scr, cub.rearrange("n t p -> n (t p)"), 1.0,
                A_all.rearrange("n t p -> n (t p)"), ALU.mult, ALU.mult,
                accum_out=out_sb[:, 1:2])
            nc.vector.tensor_reduce(out_sb[:, 2:18], A_all.rearrange("n t p -> n p t"),
                                    axis=AX.X, op=ALU.add)
            nc.sync.dma_start(out=out_d.ap(), in_=out_sb)

    nc.compile()
    return nc


def _get_nc():
    if "nc" not in _CACHE:
        _CACHE["nc"] = _build()
    return _CACHE["nc"]


def kernel(**inputs):
    import concourse.bass_utils as bass_utils

    nc = _get_nc()
    names = ["pc", "normals", "trans", "rotate", "scale", "shape_eps",
             "etas", "omegas", "assign_matrix"]
    in_maps = []
    for b in range(B):
        in_maps.append({
            k: np.ascontiguousarray(np.asarray(inputs[k][b], dtype=np.float32))
            for k in names
        })
    res = bass_utils.run_bass_kernel_spmd(nc, in_maps, core_ids=list(range(8)))

    cd_sums, cub_sums, colsums = [], [], []
    for b in range(B):
        o = np.asarray(res.results[b]["out"], dtype=np.float64)
        cd_sums.append(o[:, 0].sum())
        cub_sums.append(o[:, 1].sum())
        colsums.append(o[:, 2:18].sum(axis=0))

    cub = np.sum(cub_sums) / (B * N)
    cd = 2.0 * np.sum(cd_sums) / (B * N)
    ext_terms, sps_terms = [], []
    exist = np.asarray(inputs["exist"], dtype=np.float64)
    for b in range(B):
        gt = (colsums[b] > 24.0).astype(np.float64)
        pr = exist[b, :, 0]
        bce = -(gt * np.maximum(np.log(pr), -100.0)
                + (1 - gt) * np.maximum(np.log(1.0 - pr), -100.0))
        ext_terms.append(bce.mean())
        sps_terms.append(np.sqrt(colsums[b] / N + 0.01).mean() ** 2)
    ext = float(np.mean(ext_terms))
    sps = float(np.mean(sps_terms))
    loss = 1.0 * cub + 1.0 * cd + 0.1 * ext + 0.1 * sps
    return np.float32(loss)
